# revision 1
# baseline (speedup 1.0000x reference)
"""TRN2 Bass kernel for nn_AttentionBlock (N=4, C=256, L=4096, 4 heads, AGGR=4).

Sharding: 8 cores = (batch n, L-half). Core c handles n=c//2, query positions
l in [half*2048, (half+1)*2048). Each core computes k/v from the full
aggregated sequence of its batch (L2=1024) and produces the full output slice
out[n][:, l_half] -- no cross-core reduction needed.

The host hands each core x[n] with columns PERMUTED so the core's own query
half comes first (attention is permutation-invariant over key positions, and
the 4-wide pooling windows stay intact), so the query slice is a static
[:, 0:2048] view and is available as soon as the first DMA half lands.

Matmuls run in float32r (full PE rate, ~1.5e-4 error) except the attention
o-matmul whose operands (exp'd scores, v'^T) are bf16. Attention uses the
S^T layout (keys on partitions): the softmax denominator comes free from an
appended ones-column in v'^T; normalization is reciprocal +
partition-broadcast + multiply. o-matmuls trail the exp stream by one m-tile
so PE and ACT stay co-busy with no epilogue.
"""

import numpy as np

N, C, L = 4, 256, 4096
HEAD_DIM = 64
H = C // HEAD_DIM          # 4 heads
AGGR = 4
L2 = L // AGGR             # 1024 aggregated positions
LH = L // 2                # 2048 query positions per core
BN_EPS = 1e-5
N_CORES = 8

_CACHE = {}


def _build_program():
    import concourse.bass as bass
    import concourse.bacc as bacc
    import concourse.tile as tile
    from concourse import mybir
    from contextlib import ExitStack

    dt = mybir.dt
    f32 = dt.float32
    f32r = dt.float32r
    bf16 = dt.bfloat16
    AF = mybir.ActivationFunctionType
    Alu = mybir.AluOpType

    nc = bacc.Bacc("TRN2", debug=False, num_devices=N_CORES)

    xf_d = nc.dram_tensor("x_full", [C, L], f32, kind="ExternalInput")
    wqt_d = nc.dram_tensor("wqt", [C, C], f32, kind="ExternalInput")
    wkt_d = nc.dram_tensor("wkt", [C, C], f32, kind="ExternalInput")
    wvt_d = nc.dram_tensor("wvt", [C, C], f32, kind="ExternalInput")
    wot_d = nc.dram_tensor("wot", [C, C], f32, kind="ExternalInput")
    wat_d = nc.dram_tensor("wat", [C, C], f32, kind="ExternalInput")
    # rows: bq, bk, t(bn-folded xa bias), bo
    bp_d = nc.dram_tensor("biasp", [4, C], f32, kind="ExternalInput")
    bv_d = nc.dram_tensor("bv", [C], f32, kind="ExternalInput")
    out_d = nc.dram_tensor("out", [C, LH], f32, kind="ExternalOutput")

    with tile.TileContext(nc) as tc, ExitStack() as ctx:
        pp = ctx.enter_context(tc.tile_pool(name="persist", bufs=1))
        scr_w = ctx.enter_context(tc.tile_pool(name="scr_w", bufs=1))
        scr_p = ctx.enter_context(tc.tile_pool(name="scr_p", bufs=3))
        at_pool = ctx.enter_context(tc.tile_pool(name="at", bufs=6))
        oa_pool = ctx.enter_context(tc.tile_pool(name="oa", bufs=2))
        outp = ctx.enter_context(tc.tile_pool(name="outp", bufs=3))
        r_pool = ctx.enter_context(tc.tile_pool(name="rp", bufs=2))
        R_pool = ctx.enter_context(tc.tile_pool(name="Rp", bufs=3))

        ps_s = ctx.enter_context(tc.tile_pool(name="ps_s", bufs=3, space="PSUM"))
        ps_o = ctx.enter_context(tc.tile_pool(name="ps_o", bufs=2, space="PSUM"))

        # ---- persistent tiles ----
        xf = [pp.tile([128, L], f32, name=f"xf{ct}", tag=f"xf{ct}")
              for ct in range(2)]
        xq_r = [pp.tile([128, LH], f32r, name=f"xqr{ct}", tag=f"xqr{ct}")
                for ct in range(2)]
        q_r = [pp.tile([128, LH], bf16, name=f"qr{ct}", tag=f"qr{ct}")
               for ct in range(2)]
        k_r = [pp.tile([128, L2], bf16, name=f"kr{ct}", tag=f"kr{ct}")
               for ct in range(2)]
        xa_r = [pp.tile([128, L2], f32r, name=f"xar{ct}", tag=f"xar{ct}")
                for ct in range(2)]
        p_r = [pp.tile([128, L2], f32r, name=f"pr{ct}", tag=f"pr{ct}")
               for ct in range(2)]
        # v'^T per m-tile: 4 heads x (64 cols + ones col), bf16
        v_r = [pp.tile([128, 4 * 65], bf16, name=f"vr{mt}", tag=f"vr{mt}")
               for mt in range(8)]
        wts = {}
        for wname in ("wqt", "wkt", "wvt", "wot", "wat"):
            wts[wname] = pp.tile([128, 512], f32r, name=wname, tag=wname)
        bias_t = [pp.tile([128, 4], f32, name=f"bias{ct}", tag=f"bias{ct}")
                  for ct in range(2)]
        bv_r = pp.tile([1, C], f32r, name="bvr", tag="bvr")
        ones_r = pp.tile([1, 128], f32r, name="ones", tag="ones")

        # ---- DMAs: x on the SP queue; weights/biases on the ACT queue so
        # they land in parallel with the 4MB of x ----
        # finer x chunks so the weight DMAs can interleave on the DMA engines
        for half in range(2):
            for ct in range(2):
                for sub in range(2):
                    c0 = half * 2048 + sub * 1024
                    nc.sync.dma_start(
                        xf[ct][:, c0:c0 + 1024],
                        xf_d.ap()[ct * 128:(ct + 1) * 128, c0:c0 + 1024])
        wt_dram = {"wqt": wqt_d, "wkt": wkt_d, "wvt": wvt_d, "wot": wot_d,
                   "wat": wat_d}
        w_f = {}
        for wname in ("wat", "wkt", "wqt", "wvt", "wot"):
            wf = scr_w.tile([128, 512], f32, name=f"wf_{wname}",
                            tag=f"wf_{wname}")
            src = wt_dram[wname].ap().rearrange("(k p) o -> p k o", p=128)
            nc.scalar.dma_start(wf[:].rearrange("p (k o) -> p k o", k=2), src)
            w_f[wname] = wf
        for ct in range(2):
            nc.scalar.dma_start(
                bias_t[ct][:], bp_d.ap().rearrange("b (k p) -> k p b", p=128)[ct])
        bv_f = r_pool.tile([1, C], f32, name="bv_f", tag="bv_f", bufs=1)
        nc.scalar.dma_start(bv_f[:], bv_d.ap().rearrange("(a o) -> a o", a=1))

        # ---- gpsimd: constants, converts, query-slice copies ----
        ones_f = scr_w.tile([128, 128], f32, name="ones_f", tag="ones_f")
        nc.gpsimd.memset(ones_f[:], 1.0)
        nc.gpsimd.tensor_copy(ones_r[:], ones_f[0:1, :])
        for mt in range(8):
            nc.gpsimd.memset(
                v_r[mt][:].rearrange("p (h e) -> p h e", e=65)[:, :, 64], 1.0)
        # pre-warm the ACT exp table during the idle prefix
        warm = scr_w.tile([1, 8], f32, name="warm", tag="warm")
        nc.scalar.activation(warm[:], ones_f[0:1, 0:8], AF.Exp, scale=1.0)
        for wname in ("wat", "wkt", "wqt"):
            nc.gpsimd.tensor_copy(wts[wname][:], w_f[wname][:])
        # query-slice rounding copies on gpsimd, first pieces early (q lc0)
        for ct in range(2):
            nc.gpsimd.tensor_copy(xq_r[ct][:, 0:512], xf[ct][:, 0:512])
        nc.gpsimd.tensor_copy(wts["wvt"][:], w_f["wvt"][:])
        nc.gpsimd.tensor_copy(bv_r[:], bv_f[:])
        for ct in range(2):
            nc.gpsimd.tensor_copy(xq_r[ct][:, 512:LH], xf[ct][:, 512:LH])

        def w_block(wname, cch, ct_out):
            # lhsT block [c_in 128, c_out 128] for chunk cch, out tile ct_out
            return wts[wname][:, cch * 256 + ct_out * 128:
                              cch * 256 + ct_out * 128 + 128]

        # ---- pool quadrants: p = avg4 + max4 ----
        def emit_stt(mc, ct, a1, m1, eng=None):
            (eng or nc.vector).scalar_tensor_tensor(
                p_r[ct][:, mc * 512:(mc + 1) * 512], a1[:], 0.25, m1[:],
                Alu.mult, Alu.add)

        def pool_quadrant(mc, ct, eng, do_stt=True, sub=None):
            c0, cw = mc * 2048, 2048
            s0, sw = mc * 512, 512
            if sub is not None:
                c0, cw = c0 + sub * 1024, 1024
                s0, sw = s0 + sub * 256, 256
            xv = xf[ct][:, c0:c0 + cw].rearrange("p (m g) -> p m g", g=4)
            a1 = scr_p.tile([128, 512], f32, name="pa1", tag="pa1")
            a2 = scr_p.tile([128, 512], f32, name="pa2", tag="pa2")
            m1 = scr_p.tile([128, 512], f32, name="pm1", tag="pm1")
            m2 = scr_p.tile([128, 512], f32, name="pm2", tag="pm2")
            eng.tensor_tensor(a1[:, 0:sw], xv[:, :, 0], xv[:, :, 1], Alu.add)
            eng.tensor_tensor(a2[:, 0:sw], xv[:, :, 2], xv[:, :, 3], Alu.add)
            eng.tensor_tensor(m1[:, 0:sw], xv[:, :, 0], xv[:, :, 1], Alu.max)
            eng.tensor_tensor(m2[:, 0:sw], xv[:, :, 2], xv[:, :, 3], Alu.max)
            eng.tensor_tensor(a1[:, 0:sw], a1[:, 0:sw], a2[:, 0:sw], Alu.add)
            eng.tensor_tensor(m1[:, 0:sw], m1[:, 0:sw], m2[:, 0:sw], Alu.max)
            if do_stt:
                nc.vector.scalar_tensor_tensor(
                    p_r[ct][:, s0:s0 + sw], a1[:, 0:sw], 0.25, m1[:, 0:sw],
                    Alu.mult, Alu.add)
            return a1, m1

        # first quadrants in halves: start right after the first x chunks
        pool_quadrant(0, 0, nc.vector, sub=0)
        pool_quadrant(0, 0, nc.vector, sub=1)
        pool_quadrant(0, 1, nc.vector, sub=0)
        pool_quadrant(0, 1, nc.vector, sub=1)

        # ---- projection chunk helpers ----
        def proj_chunk(wname, src, dst, bias_col, nn2, eng):
            for ct_out in range(2):
                ps = ps_s.tile([128, 512], f32, name="ps_s", tag="ps_s")
                for cch in range(2):
                    nc.tensor.matmul(
                        ps[:], w_block(wname, cch, ct_out),
                        src[cch][:, nn2 * 512:(nn2 + 1) * 512],
                        start=(cch == 0), stop=(cch == 1))
                if eng is nc.scalar:
                    nc.scalar.add(dst[ct_out][:, nn2 * 512:(nn2 + 1) * 512],
                                  ps[:], bias_t[ct_out][:, bias_col:bias_col + 1])
                else:
                    eng.tensor_scalar(
                        dst[ct_out][:, nn2 * 512:(nn2 + 1) * 512], ps[:],
                        bias_t[ct_out][:, bias_col:bias_col + 1], None, Alu.add)

        def q_chunk(lcq, eng):
            for ct_out in range(2):
                ps = ps_s.tile([128, 512], f32, name="ps_s", tag="ps_s")
                for cch in range(2):
                    nc.tensor.matmul(
                        ps[:], w_block("wqt", cch, ct_out),
                        xq_r[cch][:, lcq * 512:(lcq + 1) * 512],
                        start=(cch == 0), stop=(cch == 1))
                if eng is nc.scalar:
                    nc.scalar.add(q_r[ct_out][:, lcq * 512:(lcq + 1) * 512],
                                  ps[:], bias_t[ct_out][:, 0:1])
                else:
                    eng.tensor_scalar(
                        q_r[ct_out][:, lcq * 512:(lcq + 1) * 512], ps[:],
                        bias_t[ct_out][:, 0:1], None, Alu.add)

        def v_block(mt, copy_eng, vpool=None):
            vpool = vpool or ps_o
            tag = "ps_o" if vpool is ps_o else "ps_s"
            pv = vpool.tile([128, C], f32, name="ps_v", tag=tag)
            for cch in range(2):
                nc.tensor.matmul(
                    pv[:], xa_r[cch][:, mt * 128:(mt + 1) * 128],
                    wts["wvt"][:, cch * 256:(cch + 1) * 256],
                    start=(cch == 0), stop=False)
            nc.tensor.matmul(pv[:], ones_r[:1, :], bv_r[:1, :],
                             start=False, stop=True)
            vv = v_r[mt][:].rearrange("p (h e) -> p h e", e=65)
            copy_eng.tensor_copy(
                vv[:, :, 0:64], pv[:].rearrange("p (h e) -> p h e", e=64))

        # ---- prefix: q lc0, xa/k chunk n0, rest of pool, v 0-3 ----
        q_chunk(0, nc.scalar)
        proj_chunk("wat", p_r, xa_r, 2, 0, nc.scalar)
        proj_chunk("wkt", xa_r, k_r, 1, 0, nc.scalar)
        # remaining pool quadrants (DVE; walrus rejects tensor ops on gpsimd)
        pool_quadrant(1, 0, nc.vector)
        for mt in range(2):
            v_block(mt, nc.vector)
        pool_quadrant(1, 1, nc.vector)
        for mt in range(2, 4):
            v_block(mt, nc.vector)
        nc.gpsimd.tensor_copy(wts["wot"][:], w_f["wot"][:])

        # ---- attention: o-matmuls lag exp by one m-tile; the previous
        # iteration's softmax-normalize and Wo conv are emitted inside the
        # next iteration's S/exp stream so they overlap it ----
        oa_tiles = {}

        def norm_prev(state):
            lc, hp, po = state
            oa = oa_tiles[lc]
            for h2 in range(2):
                r_t = r_pool.tile([1, 512], f32, name="r", tag="r")
                nc.vector.reciprocal(r_t[:], po[h2][64:65, :])
                R_t = R_pool.tile([64, 512], f32, name="R", tag="R")
                nc.gpsimd.partition_broadcast(R_t[:], r_t[:], channels=64)
                nc.vector.tensor_tensor(
                    oa[hp][h2 * 64:(h2 + 1) * 64, :], po[h2][0:64, :],
                    R_t[:], Alu.mult)

        def wo_prev(state):
            lc, hp, po = state
            if hp != 1:
                return
            oa = oa_tiles[lc]
            for ct_out in range(2):
                psW = ps_s.tile([128, 512], f32, name="ps_s", tag="ps_s")
                for cch in range(2):
                    nc.tensor.matmul(
                        psW[:], w_block("wot", cch, ct_out), oa[cch][:],
                        start=(cch == 0), stop=(cch == 1))
                out_t = outp.tile([128, 512], f32, name="out", tag="out")
                nc.vector.tensor_scalar(out_t[:], psW[:],
                                        bias_t[ct_out][:, 3:4], None,
                                        Alu.add)
                nc.sync.dma_start(
                    out_d.ap()[ct_out * 128:(ct_out + 1) * 128,
                               lc * 512:(lc + 1) * 512], out_t[:])
            del oa_tiles[lc]

        # pending o-matmul FIFO: one pair popped per (S, exp) step, crossing
        # iteration boundaries so PE never waits on the last exp of an iter
        pending = []

        def emit_iter(lc, hp, prev_state, mid_hook=None):
            if hp == 0:
                oa_tiles[lc] = [
                    oa_pool.tile([128, 512], f32r, name=f"oa{ct}",
                                 tag=f"oa{ct}") for ct in range(2)]
            po = [ps_o.tile([65, 512], f32, name="ps_o", tag="ps_o")
                  for _ in range(2)]

            def make_o(mt, at):
                def emit():
                    for h2 in range(2):
                        h = 2 * hp + h2
                        nc.tensor.matmul(
                            po[h2][:], v_r[mt][:, h * 65:h * 65 + 65],
                            at[:, h2 * 512:(h2 + 1) * 512],
                            start=(mt == 0), stop=(mt == 7))
                return emit

            for mt in range(8):
                if mt == 4 and mid_hook is not None:
                    mid_hook()
                ps = ps_s.tile([128, L2], f32, name="ps_s", tag="ps_s")
                for h2 in range(2):
                    nc.tensor.matmul(
                        ps[:, h2 * 512:(h2 + 1) * 512],
                        k_r[hp][h2 * 64:(h2 + 1) * 64, mt * 128:(mt + 1) * 128],
                        q_r[hp][h2 * 64:(h2 + 1) * 64, lc * 512:(lc + 1) * 512],
                        start=True, stop=True)
                at = at_pool.tile([128, 1024], bf16, name="at", tag="at")
                nc.scalar.activation(at[:], ps[:], AF.Exp, scale=0.125)
                pending.append(make_o(mt, at))
                # with the deeper o-FIFO, the previous iteration's last
                # o-matmul is popped during step mt1, so its normalize may
                # be emitted no earlier than mt2 (else it misses mt7)
                if mt == 2 and prev_state is not None:
                    norm_prev(prev_state)
                if mt == 5 and prev_state is not None:
                    wo_prev(prev_state)
                if len(pending) >= 3:
                    pending.pop(0)()
            return (lc, hp, po)

        # iteration (0,0) with the n1 projections + v 4-7 emitted mid-stream
        def mid():
            proj_chunk("wat", p_r, xa_r, 2, 1, nc.scalar)
            proj_chunk("wkt", xa_r, k_r, 1, 1, nc.scalar)
            for mt in range(4, 8):
                v_block(mt, nc.vector, vpool=ps_s)

        state = emit_iter(0, 0, None, mid_hook=mid)
        q_after = {(0, 1): 1, (1, 0): 2, (1, 1): 3}
        for lc, hp in [(0, 1), (1, 0), (1, 1), (2, 0), (2, 1), (3, 0), (3, 1)]:
            state = emit_iter(lc, hp, state)
            lcq = q_after.get((lc, hp))
            if lcq:
                q_chunk(lcq, nc.vector)
        while pending:
            pending.pop(0)()
        norm_prev(state)
        wo_prev(state)

    nc.compile()
    return nc


def _get_program():
    if "nc" not in _CACHE:
        _CACHE["nc"] = _build_program()
    return _CACHE["nc"]


def kernel(x, Wq, bq, Wk, bk, Wv, bv, Wo, bo, Wa,
           g1, b1, m1, v1, g2, b2, m2, v2):
    from concourse import bass_utils

    nc = _get_program()

    x = np.asarray(x, dtype=np.float32)
    # fold both eval-mode BNs into a per-channel affine: xa = s*(Wa@p) + t
    s1 = np.asarray(g1) / np.sqrt(np.asarray(v1) + BN_EPS)
    t1 = np.asarray(b1) - np.asarray(m1) * s1
    s2 = np.asarray(g2) / np.sqrt(np.asarray(v2) + BN_EPS)
    t2 = np.asarray(b2) - np.asarray(m2) * s2
    s = (s1 * s2).astype(np.float32)
    t = (t1 * s2 + t2).astype(np.float32)

    wat = (np.asarray(Wa) * s[:, None]).astype(np.float32).T.copy()
    wqt = np.asarray(Wq, dtype=np.float32).T.copy()
    wkt = np.asarray(Wk, dtype=np.float32).T.copy()
    wvt = np.asarray(Wv, dtype=np.float32).T.copy()
    wot = np.asarray(Wo, dtype=np.float32).T.copy()
    biasp = np.stack([np.asarray(bq), np.asarray(bk), t,
                      np.asarray(bo)]).astype(np.float32)
    bvv = np.asarray(bv, dtype=np.float32)

    shared = {"wqt": wqt, "wkt": wkt, "wvt": wvt, "wot": wot, "wat": wat,
              "biasp": biasp, "bv": bvv}
    in_maps = []
    for c in range(N_CORES):
        n, half = c // 2, c % 2
        m = dict(shared)
        xs = x[n]
        if half == 0:
            m["x_full"] = np.ascontiguousarray(xs)
        else:
            # core's own query half first; key order is irrelevant
            # (pool windows intact, attention permutation-invariant)
            m["x_full"] = np.concatenate([xs[:, LH:], xs[:, :LH]], axis=1)
        in_maps.append(m)

    res = bass_utils.run_bass_kernel_spmd(nc, in_maps,
                                          core_ids=list(range(N_CORES)))
    out = np.empty((N, C, L), np.float32)
    for c in range(N_CORES):
        n, half = c // 2, c % 2
        out[n][:, half * LH:(half + 1) * LH] = res.results[c]["out"]
    return out

